# revision 17
# baseline (speedup 1.0000x reference)
# ABCNN forward kernel for 8 TRN2 NeuronCores — pure data parallel.
# Shards batch (128 -> 16/core), replicates weights, one SPMD Bass graph.
#
# Per-core layout convention: feature/channel dim on SBUF partitions,
# (sample, position) on the free dim. All input transposes done on HOST.
#
# conv = sum over (tap k, in-chunk cc, input-half i) of 128x128 stationary
# weight matmuls streaming padded activations; attention distance matrices
# via per-sample matmuls with an augmented K=2 rank-1 matmul adding the
# -(n1_i + n2_j)/2 terms; row/col sums + broadcasts via ones-matmuls.

import numpy as np
import ml_dtypes

import concourse.bass as bass
import concourse.mybir as mybir
import concourse.tile as tile
from concourse import bacc
from concourse.bass_utils import run_bass_kernel_spmd

B, S, D0, H, KK, C = 128, 64, 768, 256, 7, 3
NCORES = 8
BL = B // NCORES          # 16 samples per core
S2 = S + KK - 1           # 70 conv output positions
SP = S + 2 * (KK - 1)     # 76 padded length
EPS = 1e-12
F32 = mybir.dt.float32

USE_BF16 = True
CDT = mybir.dt.bfloat16 if USE_BF16 else F32
NPDT = ml_dtypes.bfloat16 if USE_BF16 else np.float32

# conv moving-operand sample groups (N = ns*70 <= 512)
CONV_GROUPS = [(0, 7), (7, 14), (14, 16)]
# splits of a width-1024 free axis (n-chains etc)
SEG1024 = [(0, 512), (512, 1024)]
# splits of width-1120 (= 16*70) free axis
SEG1120 = [(0, 490), (490, 980), (980, 1120)]

AF = mybir.ActivationFunctionType
ALU = mybir.AluOpType


import os
STAGES = int(os.environ.get("KERNEL_STAGES", "8"))


class _StageCut(Exception):
    pass


def build_graph():
    nc = bacc.Bacc()

    x1d = nc.declare_dram_parameter("x1", [D0, BL * SP], CDT, isOutput=False)
    x2d = nc.declare_dram_parameter("x2", [D0, BL * SP], CDT, isOutput=False)
    w1d = nc.declare_dram_parameter("w1", [2, KK, 2, D0, 128], CDT, isOutput=False)
    w2d = nc.declare_dram_parameter("w2", [2, KK, 2, H, 128], CDT, isOutput=False)
    aw1d = nc.declare_dram_parameter("aw1", [S, D0], CDT, isOutput=False)
    aw2d = nc.declare_dram_parameter("aw2", [S, H], CDT, isOutput=False)
    b1d = nc.declare_dram_parameter("b1", [2, 128, 1], F32, isOutput=False)
    b2d = nc.declare_dram_parameter("b2", [2, 128, 1], F32, isOutput=False)
    owbd = nc.declare_dram_parameter("owb", [4, C], CDT, isOutput=False)
    cvecd = nc.declare_dram_parameter("cvec", [2, 4], F32, isOutput=False)
    outd = nc.declare_dram_parameter("out", [BL, C], F32, isOutput=True)

    with tile.TileContext(nc) as tc:
        with (
            tc.tile_pool(name="consts", bufs=1) as consts,
            tc.tile_pool(name="pads", bufs=26) as padsp,
            tc.tile_pool(name="w1p", bufs=28) as w1p,
            tc.tile_pool(name="w2p", bufs=28) as w2p,
            tc.tile_pool(name="sqp", bufs=6) as sqp,
            tc.tile_pool(name="attb", bufs=2) as attbp,
            tc.tile_pool(name="augp", bufs=4) as augp,
            tc.tile_pool(name="lcp", bufs=6) as lcp,
            tc.tile_pool(name="wtree", bufs=6) as wtreep,
            tc.tile_pool(name="lop", bufs=24) as lop,
            tc.tile_pool(name="smallp", bufs=8) as smallp,
            tc.tile_pool(name="distp", bufs=3) as distp,
            tc.tile_pool(name="psp", bufs=7, space="PSUM") as psp,
        ):
          try:
            # ---------------- constants ----------------
            ones2 = consts.tile([128, 2], CDT, tag="c_ones2")
            nc.vector.memset(ones2, 1.0)
            ones70x128 = consts.tile([S2, 128], CDT, tag="c_ones70")
            nc.vector.memset(ones70x128, 1.0)
            onesf = consts.tile([1, BL], CDT, tag="c_onesf")
            nc.vector.memset(onesf, 1.0)
            cvecsb = consts.tile([2, 4], F32, tag="c_cvec")
            nc.sync.dma_start(out=cvecsb, in_=cvecd[:])
            ecols = []
            for p in range(3):
                e = consts.tile([128, 3], CDT, tag=f"c_e{p}")
                nc.vector.memset(e, 0.0)
                nc.vector.memset(e[:, p : p + 1], 1.0)
                ecols.append(e)

            aw1sb = consts.tile([S, D0], CDT, tag="c_aw1")
            nc.sync.dma_start(out=aw1sb, in_=aw1d[:])
            aw2sb = consts.tile([S, H], CDT, tag="c_aw2")
            nc.sync.dma_start(out=aw2sb, in_=aw2d[:])
            b1t = [
                consts.tile([128, 1], F32, tag=f"c_b1_{hc}", name=f"b1t{hc}")
                for hc in range(2)
            ]
            b2t = [
                consts.tile([128, 1], F32, tag=f"c_b2_{hc}", name=f"b2t{hc}")
                for hc in range(2)
            ]
            for hc in range(2):
                nc.sync.dma_start(out=b1t[hc], in_=b1d[hc, :, :])
                nc.sync.dma_start(out=b2t[hc], in_=b2d[hc, :, :])
            epsb = consts.tile([128, 1], F32, tag="c_epsb")
            nc.vector.memset(epsb, EPS)
            owsb = consts.tile([3, C], CDT, tag="c_ow")
            nc.sync.dma_start(out=owsb, in_=owbd[0:3, :])
            obsb = consts.tile([1, C], CDT, tag="c_ob")
            nc.sync.dma_start(out=obsb, in_=owbd[3:4, :])

            # ---------------- load inputs into padded layout ----------------
            def make_pads(n_chunks, tagpfx, zero=True):
                tiles = []
                for cc in range(n_chunks):
                    t = padsp.tile([128, BL * SP], CDT, tag="pad",
                                   name=f"pad_{tagpfx}{cc}")
                    tiles.append(t)
                if zero:
                    zero_borders(tiles)
                return tiles

            def zero_borders(tiles):
                # interiors are fully overwritten; only the K-1 halo columns
                # around each sample need zeros
                for t in tiles:
                    tv = t.rearrange("p (s w) -> p s w", w=SP)
                    nc.vector.memset(tv[:, 0:1, 0 : KK - 1], 0.0)
                    nc.vector.memset(
                        t[:, (BL - 1) * SP + KK - 1 + S : BL * SP], 0.0
                    )
                    mid = bass.AP(
                        tensor=t.tensor,
                        offset=t.offset + KK - 1 + S,
                        ap=[t.ap[0], [SP, BL - 1], [1, 2 * (KK - 1)]],
                    )
                    nc.vector.memset(mid, 0.0)

            # x1/x2 arrive host-pre-padded: plain full-tile DMAs, no memset dep
            x1pad = make_pads(6, "x1", zero=False)
            x2pad = make_pads(6, "x2", zero=False)

            for pads, src in ((x1pad, x1d), (x2pad, x2d)):
                for cc in range(6):
                    nc.sync.dma_start(
                        out=pads[cc],
                        in_=src[cc * 128 : (cc + 1) * 128, :],
                    )

            # unpadded view helper: [128, ns, S] slice of a pad tile
            def unpad(t, s0=0, s1=BL, width=S):
                return t.rearrange("p (s w) -> p s w", w=SP)[
                    :, s0:s1, KK - 1 : KK - 1 + width
                ]

            # ---------------- LO0 / RO0 (sums over positions) ----------------
            LO0 = []
            RO0 = []
            for pads, acc in ((x1pad, LO0), (x2pad, RO0)):
                for cc in range(6):
                    t = lop.tile([128, BL], F32, tag="lo")
                    nc.vector.reduce_sum(t, unpad(pads[cc]), axis=mybir.AxisListType.X)
                    acc.append(t)

            # ---------------- generic attention stage ----------------
            # xun(cc, s0, s1) -> AP [P, (s1-s0), W]; returns attA [W, BL*W] (=[i,j])
            # and attB (=[j,i]) tiles in CDT.
            def attention(n_cc, W, xun, yun, segs, tagpfx):
                NW = BL * W
                # squares
                sqx = []
                sqy = []
                for un, acc in ((xun, sqx), (yun, sqy)):
                    for cc in range(n_cc):
                        t = sqp.tile([128, NW], CDT, tag="sq")
                        u = un(cc, 0, BL)
                        nc.vector.tensor_mul(
                            t.rearrange("p (s w) -> p s w", w=W), u, u
                        )
                        sqx_ = acc.append(t)
                # n-chains: psum [2, seg], both rows = n
                def nchain(sqtiles):
                    pss = []
                    for (o0, o1) in segs:
                        ps = psp.tile([2, o1 - o0], F32, tag="ps")
                        for cc in range(n_cc):
                            nc.tensor.matmul(
                                ps,
                                lhsT=ones2,
                                rhs=sqtiles[cc][:, o0:o1],
                                start=(cc == 0),
                                stop=(cc == n_cc - 1),
                            )
                        pss.append(ps)
                    return pss
                n1ps = nchain(sqx)
                n2ps = nchain(sqy)
                # aug tiles
                aug_al = augp.tile([2, NW], CDT, tag="aug")  # (n1, 1)
                aug_ar = augp.tile([2, NW], CDT, tag="aug")  # (-1/2, -n2/2)
                aug_bl = augp.tile([2, NW], CDT, tag="aug")  # (n2, 1)
                aug_br = augp.tile([2, NW], CDT, tag="aug")  # (-1/2, -n1/2)
                # rows built in one dual-op tensor_scalar per seg:
                # out = in * mulvec + addvec, with per-partition vectors
                # keep = (1,0)+(0,1); nhalf = (0,-1/2)+(-1/2,0)
                c_keep_m, c_keep_a = cvecsb[:, 0:1], cvecsb[:, 1:2]
                c_nh_m, c_nh_a = cvecsb[:, 2:3], cvecsb[:, 3:4]
                for si, (o0, o1) in enumerate(segs):
                    nc.scalar.activation(
                        aug_al[:, o0:o1], n1ps[si], AF.Identity,
                        bias=c_keep_a, scale=c_keep_m,
                    )
                    nc.scalar.activation(
                        aug_br[:, o0:o1], n1ps[si], AF.Identity,
                        bias=c_nh_a, scale=c_nh_m,
                    )
                    nc.scalar.activation(
                        aug_bl[:, o0:o1], n2ps[si], AF.Identity,
                        bias=c_keep_a, scale=c_keep_m,
                    )
                    nc.scalar.activation(
                        aug_ar[:, o0:o1], n2ps[si], AF.Identity,
                        bias=c_nh_a, scale=c_nh_m,
                    )
                # per-sample dist chains -> batched nonlinearity
                attA = attbp.tile([W, NW], CDT, tag="attb")
                attB = attbp.tile([W, NW], CDT, tag="attb")
                distA = distp.tile([W, NW], F32, tag="dist")
                distB = distp.tile([W, NW], F32, tag="dist")
                for s in range(BL):
                    sl = slice(s * W, (s + 1) * W)
                    for (lef, rig, al, ar, dst) in (
                        (xun, yun, aug_al, aug_ar, distA),
                        (yun, xun, aug_bl, aug_br, distB),
                    ):
                        ps = psp.tile([W, W], F32, tag="ps")
                        for cc in range(n_cc):
                            nc.tensor.matmul(
                                ps,
                                lhsT=lef(cc, s, s + 1),
                                rhs=rig(cc, s, s + 1),
                                start=(cc == 0),
                                stop=False,
                            )
                        nc.tensor.matmul(
                            ps, lhsT=al[:, sl], rhs=ar[:, sl], start=False, stop=True
                        )
                        # dist^2 = max(-2 * ps, 0), evacuated batched
                        nc.vector.tensor_scalar(
                            dst[:, sl], ps, -2.0, 0.0, op0=ALU.mult, op1=ALU.max
                        )
                for (dst, dist) in ((attA, distA), (attB, distB)):
                    t2 = distp.tile([W, NW], F32, tag="dist")
                    nc.scalar.activation(t2, dist, AF.Sqrt, bias=epsb[:W, :])
                    nc.vector.tensor_scalar_add(t2, t2, 1.0)
                    with nc.allow_low_precision(reason="att stored bf16"):
                        nc.vector.reciprocal(dst, t2)
                return attA, attB

            x1un = lambda cc, s0, s1: unpad(x1pad[cc], s0, s1)
            x2un = lambda cc, s0, s1: unpad(x2pad[cc], s0, s1)
            if STAGES >= 2:
                attA1, attB1 = attention(6, S, x1un, x2un, SEG1024, "a1")

            # ---------------- x1a / x2a (aW1-weighted) ----------------
            x1apad = make_pads(6, "x1a")
            x2apad = make_pads(6, "x2a")
            # x1a[d, (s,i)] = sum_j attB1[j, (s,i)] * aW1[j, d]
            for attX, dstpads, awsb, ncc in (
                ((attB1, x1apad, aw1sb, 6),
                 (attA1, x2apad, aw1sb, 6)) if STAGES >= 3 else ()
            ):
                for cc in range(ncc):
                    for half in range(2):
                        ps = psp.tile([128, 8 * S], F32, tag="ps")
                        nc.tensor.matmul(
                            ps,
                            lhsT=awsb[:, cc * 128 : (cc + 1) * 128],
                            rhs=attX[:, half * 8 * S : (half + 1) * 8 * S],
                            start=True,
                            stop=True,
                        )
                        dst = unpad(dstpads[cc], half * 8, (half + 1) * 8)
                        nc.scalar.activation(dst, ps, AF.Copy)

            # ---------------- load conv weights ----------------
            def load_w(wd, n_cc, pool):
                tiles = {}
                for hc in range(2):
                    for k in range(KK):
                        for i in range(2):
                            t = pool.tile([128, n_cc * 128], CDT, tag="w")
                            nc.sync.dma_start(
                                out=t.rearrange("p (c h) -> p c h", h=128),
                                in_=wd[hc, k, i, :, :].rearrange(
                                    "(c p) h -> p c h", p=128
                                ),
                            )
                            tiles[(hc, k, i)] = t
                return tiles

            w1t = load_w(w1d, 6, w1p) if STAGES >= 4 else None

            # ---------------- conv layer (generic) ----------------
            def conv(n_cc, ipads, apads, wt, bt, tagpfx):
                # ipads/apads: [side][cc] pad tiles; returns lc[side][hc]
                lc = [[None, None], [None, None]]
                for hc in range(2):
                    pss = {}
                    for side in range(2):
                        for gi, (s0, s1) in enumerate(CONV_GROUPS):
                            pss[(side, gi)] = psp.tile(
                                [128, (s1 - s0) * S2], F32, tag="ps",
                                name=f"convps{side}_{gi}",
                            )
                    nstat = KK * 2 * n_cc
                    idx = 0
                    for k in range(KK):
                        for i in range(2):
                            for cc in range(n_cc):
                                stat = wt[(hc, k, i)].rearrange(
                                    "p (c h) -> p c h", h=128
                                )[:, cc, :]
                                first = idx == 0
                                last = idx == nstat - 1
                                idx += 1
                                for side in range(2):
                                    src = (ipads if i == 0 else apads)[side][cc]
                                    srcv = src.rearrange("p (s w) -> p s w", w=SP)
                                    for gi, (s0, s1) in enumerate(CONV_GROUPS):
                                        nc.tensor.matmul(
                                            pss[(side, gi)],
                                            lhsT=stat,
                                            rhs=srcv[:, s0:s1, k : k + S2],
                                            start=first,
                                            stop=last,
                                        )
                    for side in range(2):
                        t = lcp.tile([128, BL * S2], CDT, tag="lc")
                        lc[side][hc] = t
                        for gi, (s0, s1) in enumerate(CONV_GROUPS):
                            nc.scalar.activation(
                                t[:, s0 * S2 : s1 * S2],
                                pss[(side, gi)],
                                AF.Tanh,
                                bias=bt[hc],
                            )
                return lc

            lc1 = (
                conv(6, [x1pad, x2pad], [x1apad, x2apad], w1t, b1t, "c1")
                if STAGES >= 4 else None
            )

            # ---------------- layer-1 att2 + pooling ----------------
            lcun = lambda tiles: (  # noqa: E731
                lambda cc, s0, s1: tiles[cc].rearrange("p (s w) -> p s w", w=S2)[
                    :, s0:s1, :
                ]
            )
            if STAGES < 5:
                raise _StageCut
            att2A, att2B = attention(
                2, S2, lcun(lc1[0]), lcun(lc1[1]), SEG1120, "a2"
            )

            # la (multiplies lc, from att2B) / ra (multiplies rc, from att2A),
            # broadcast over 128 partitions via all-ones stationary
            labc = lcp.tile([128, BL * S2], CDT, tag="lc")
            rabc = lcp.tile([128, BL * S2], CDT, tag="lc")
            for att2X, dst in ((att2B, labc), (att2A, rabc)):
                for (o0, o1) in SEG1120:
                    ps = psp.tile([128, o1 - o0], F32, tag="ps")
                    nc.tensor.matmul(
                        ps, lhsT=ones70x128, rhs=att2X[:, o0:o1], start=True, stop=True
                    )
                    nc.scalar.activation(dst[:, o0:o1], ps, AF.Copy)

            # w_pool -> LI1/RI1 padded for layer 2; lap/rap sums
            li1pad = make_pads(2, "li1")
            ri1pad = make_pads(2, "ri1")
            lap1, rap1 = [], []
            for side, (bc, dstpads, acc) in enumerate(
                ((labc, li1pad, lap1), (rabc, ri1pad, rap1))
            ):
                for hc in range(2):
                    lct = lc1[side][hc]
                    w = wtreep.tile([128, BL * S2], CDT, tag="wt")
                    nc.vector.tensor_mul(w, lct, bc)
                    wv = w.rearrange("p (s w) -> p s w", w=S2)
                    t1 = wtreep.tile([128, BL * S2], CDT, tag="wt")
                    t1v = t1.rearrange("p (s w) -> p s w", w=S2)
                    nc.vector.tensor_add(
                        t1v[:, :, 0:69], wv[:, :, 0:69], wv[:, :, 1:70]
                    )
                    t2 = wtreep.tile([128, BL * S2], CDT, tag="wt")
                    t2v = t2.rearrange("p (s w) -> p s w", w=S2)
                    nc.vector.tensor_add(
                        t2v[:, :, 0:67], t1v[:, :, 0:67], t1v[:, :, 2:69]
                    )
                    t3 = wtreep.tile([128, BL * S2], CDT, tag="wt")
                    t3v = t3.rearrange("p (s w) -> p s w", w=S2)
                    nc.vector.tensor_add(
                        t3v[:, :, 0:64], t2v[:, :, 0:64], t1v[:, :, 4:68]
                    )
                    dst = dstpads[hc].rearrange("p (s w) -> p s w", w=SP)[
                        :, :, KK - 1 : KK - 1 + S
                    ]
                    nc.vector.tensor_add(dst, t3v[:, :, 0:64], wv[:, :, 6:70])
                    # lap/rap: sum over the 70 positions
                    t = lop.tile([128, BL], F32, tag="lo")
                    nc.vector.reduce_sum(
                        t,
                        lct.rearrange("p (s w) -> p s w", w=S2),
                        axis=mybir.AxisListType.X,
                    )
                    acc.append(t)

            # ---------------- layer 2 ----------------
            if STAGES < 6:
                raise _StageCut
            li1un = lambda cc, s0, s1: unpad(li1pad[cc], s0, s1)
            ri1un = lambda cc, s0, s1: unpad(ri1pad[cc], s0, s1)
            attA2, attB2 = attention(2, S, li1un, ri1un, SEG1024, "aL2")

            la2pad = make_pads(2, "la2")
            ra2pad = make_pads(2, "ra2")
            for attX, dstpads in ((attB2, la2pad), (attA2, ra2pad)):
                for cc in range(2):
                    for half in range(2):
                        ps = psp.tile([128, 8 * S], F32, tag="ps")
                        nc.tensor.matmul(
                            ps,
                            lhsT=aw2sb[:, cc * 128 : (cc + 1) * 128],
                            rhs=attX[:, half * 8 * S : (half + 1) * 8 * S],
                            start=True,
                            stop=True,
                        )
                        nc.scalar.activation(
                            unpad(dstpads[cc], half * 8, (half + 1) * 8), ps, AF.Copy
                        )

            if STAGES < 7:
                raise _StageCut
            w2t = load_w(w2d, 2, w2p)
            lc2 = conv(2, [li1pad, ri1pad], [la2pad, ra2pad], w2t, b2t, "c2")

            lap2, rap2 = [], []
            for side, acc in ((0, lap2), (1, rap2)):
                for hc in range(2):
                    t = lop.tile([128, BL], F32, tag="lo")
                    nc.vector.reduce_sum(
                        t,
                        lc2[side][hc].rearrange("p (s w) -> p s w", w=S2),
                        axis=mybir.AxisListType.X,
                    )
                    acc.append(t)

            # ---------------- cosine sims + output ----------------
            if STAGES < 8:
                raise _StageCut
            ps_cos = psp.tile([3, 3 * BL], F32, tag="ps")
            pairs = [(LO0, RO0), (lap1, rap1), (lap2, rap2)]
            nmm = sum(len(L) for L, _ in pairs)
            mi = 0
            for p, (Ls, Rs) in enumerate(pairs):
                for cc in range(len(Ls)):
                    mov = smallp.tile([128, 3 * BL], CDT, tag="cosmov")
                    nc.vector.tensor_mul(mov[:, 0:BL], Ls[cc], Rs[cc])
                    nc.vector.tensor_mul(mov[:, BL : 2 * BL], Ls[cc], Ls[cc])
                    nc.vector.tensor_mul(mov[:, 2 * BL : 3 * BL], Rs[cc], Rs[cc])
                    nc.tensor.matmul(
                        ps_cos,
                        lhsT=ecols[p],
                        rhs=mov,
                        start=(mi == 0),
                        stop=(mi == nmm - 1),
                    )
                    mi += 1
            sq = smallp.tile([3, 2 * BL], F32, tag="costail")
            nc.scalar.activation(sq, ps_cos[:, BL : 3 * BL], AF.Sqrt)
            den = smallp.tile([3, BL], F32, tag="costail")
            nc.vector.tensor_mul(den, sq[:, 0:BL], sq[:, BL : 2 * BL])
            den2 = smallp.tile([3, BL], F32, tag="costail")
            nc.vector.tensor_scalar_add(den2, den, EPS)
            rec = smallp.tile([3, BL], F32, tag="costail")
            nc.vector.reciprocal(rec, den2)
            cosv = smallp.tile([3, BL], CDT, tag="cosv")
            nc.vector.tensor_mul(cosv, ps_cos[:, 0:BL], rec)
            ps_out = psp.tile([BL, C], F32, tag="ps")
            nc.tensor.matmul(ps_out, lhsT=cosv, rhs=owsb, start=True, stop=False)
            nc.tensor.matmul(ps_out, lhsT=onesf, rhs=obsb, start=False, stop=True)
            outsb = smallp.tile([BL, C], F32, tag="outsb")
            nc.vector.tensor_copy(outsb, ps_out)
            nc.sync.dma_start(out=outd[:], in_=outsb)
          except _StageCut:
            dummy = smallp.tile([BL, C], F32, tag="outsb", name="dummy_out")
            nc.vector.memset(dummy, 0.0)
            nc.sync.dma_start(out=outd[:], in_=dummy)

    nc.compile()
    return nc


_GRAPH = None


def get_graph():
    global _GRAPH
    if _GRAPH is None:
        _GRAPH = build_graph()
    return _GRAPH


def make_in_maps(premise, hypothesis, aW1, conv1_w, conv1_b, aW2, conv2_w,
                 conv2_b, out_w, out_b):
    premise = np.asarray(premise, np.float32)
    hypothesis = np.asarray(hypothesis, np.float32)
    w1r = np.ascontiguousarray(
        np.asarray(conv1_w, np.float32)
        .reshape(D0, KK, 2, 2, 128)
        .transpose(3, 1, 2, 0, 4)
    ).astype(NPDT)  # [hc, K, i, D0, 128]
    w2r = np.ascontiguousarray(
        np.asarray(conv2_w, np.float32)
        .reshape(H, KK, 2, 2, 128)
        .transpose(3, 1, 2, 0, 4)
    ).astype(NPDT)
    aw1 = np.asarray(aW1, np.float32).astype(NPDT)
    aw2 = np.asarray(aW2, np.float32).astype(NPDT)
    b1 = np.asarray(conv1_b, np.float32).reshape(2, 128, 1)
    b2 = np.asarray(conv2_b, np.float32).reshape(2, 128, 1)
    cvec = np.array([[1.0, 0.0, 0.0, -0.5], [0.0, 1.0, -0.5, 0.0]], np.float32)
    owb = np.concatenate(
        [np.asarray(out_w, np.float32), np.asarray(out_b, np.float32)[None, :]], 0
    ).astype(NPDT)
    in_maps = []
    for i in range(NCORES):
        sl = slice(i * BL, (i + 1) * BL)
        def prep_x(arr):
            xp = np.zeros((BL, SP, D0), np.float32)
            xp[:, KK - 1 : KK - 1 + S, :] = arr[sl]
            return np.ascontiguousarray(
                xp.transpose(2, 0, 1).reshape(D0, BL * SP)
            ).astype(NPDT)

        x1 = prep_x(premise)
        x2 = prep_x(hypothesis)
        in_maps.append(
            dict(x1=x1, x2=x2, w1=w1r, w2=w2r, aw1=aw1, aw2=aw2, b1=b1, b2=b2,
                 owb=owb, cvec=cvec)
        )
    return in_maps


def run(in_maps, trace=False):
    nc = get_graph()
    return run_bass_kernel_spmd(nc, in_maps, list(range(NCORES)), trace=trace)


def kernel(**inputs):
    in_maps = make_in_maps(**inputs)
    res = run(in_maps, trace=False)
    out = np.concatenate([res.results[i]["out"] for i in range(NCORES)], axis=0)
    return np.asarray(out, np.float32)


# revision 19
# speedup vs baseline: 1.2877x; 1.2877x over previous
# ABCNN forward kernel for 8 TRN2 NeuronCores — pure data parallel.
# Shards batch (128 -> 16/core), replicates weights, one SPMD Bass graph.
#
# Per-core layout convention: feature/channel dim on SBUF partitions,
# (sample, position) on the free dim. All input transposes done on HOST.
#
# conv = sum over (tap k, in-chunk cc, input-half i) of 128x128 stationary
# weight matmuls streaming padded activations; attention distance matrices
# via per-sample matmuls with an augmented K=2 rank-1 matmul adding the
# -(n1_i + n2_j)/2 terms; row/col sums + broadcasts via ones-matmuls.

import numpy as np
import ml_dtypes

import concourse.bass as bass
import concourse.mybir as mybir
import concourse.tile as tile
from concourse import bacc
from concourse.bass_utils import run_bass_kernel_spmd

B, S, D0, H, KK, C = 128, 64, 768, 256, 7, 3
NCORES = 8
BL = B // NCORES          # 16 samples per core
S2 = S + KK - 1           # 70 conv output positions
SP = S + 2 * (KK - 1)     # 76 padded length
EPS = 1e-12
F32 = mybir.dt.float32

USE_BF16 = True
CDT = mybir.dt.bfloat16 if USE_BF16 else F32
NPDT = ml_dtypes.bfloat16 if USE_BF16 else np.float32

# conv moving-operand sample groups (N = ns*70 <= 512)
CONV_GROUPS = [(0, 7), (7, 14), (14, 16)]
# splits of a width-1024 free axis (n-chains etc)
SEG1024 = [(0, 512), (512, 1024)]
# splits of width-1120 (= 16*70) free axis
SEG1120 = [(0, 490), (490, 980), (980, 1120)]

AF = mybir.ActivationFunctionType
ALU = mybir.AluOpType


import os
STAGES = int(os.environ.get("KERNEL_STAGES", "8"))


class _StageCut(Exception):
    pass


def build_graph():
    nc = bacc.Bacc()

    x1d = nc.declare_dram_parameter("x1", [D0, BL * SP], CDT, isOutput=False)
    x2d = nc.declare_dram_parameter("x2", [D0, BL * SP], CDT, isOutput=False)
    w1d = nc.declare_dram_parameter("w1", [2, KK, 2, D0, 128], CDT, isOutput=False)
    w2d = nc.declare_dram_parameter("w2", [2, KK, 2, H, 128], CDT, isOutput=False)
    aw1d = nc.declare_dram_parameter("aw1", [S, D0], CDT, isOutput=False)
    aw2d = nc.declare_dram_parameter("aw2", [S, H], CDT, isOutput=False)
    b1d = nc.declare_dram_parameter("b1", [2, 128, 1], F32, isOutput=False)
    b2d = nc.declare_dram_parameter("b2", [2, 128, 1], F32, isOutput=False)
    owbd = nc.declare_dram_parameter("owb", [4, C], CDT, isOutput=False)
    cvecd = nc.declare_dram_parameter("cvec", [2, 4], F32, isOutput=False)
    outd = nc.declare_dram_parameter("out", [BL, C], F32, isOutput=True)

    with tile.TileContext(nc) as tc:
        with (
            tc.tile_pool(name="consts", bufs=1) as consts,
            tc.tile_pool(name="pads", bufs=26) as padsp,
            tc.tile_pool(name="w1p", bufs=28) as w1p,
            tc.tile_pool(name="w2p", bufs=28) as w2p,
            tc.tile_pool(name="sqp", bufs=6) as sqp,
            tc.tile_pool(name="attb", bufs=2) as attbp,
            tc.tile_pool(name="augp", bufs=4) as augp,
            tc.tile_pool(name="lcp", bufs=6) as lcp,
            tc.tile_pool(name="wtree", bufs=6) as wtreep,
            tc.tile_pool(name="lop", bufs=24) as lop,
            tc.tile_pool(name="smallp", bufs=8) as smallp,
            tc.tile_pool(name="distp", bufs=6) as distp,
            tc.tile_pool(name="psp", bufs=7, space="PSUM") as psp,
        ):
          try:
            # ---------------- constants ----------------
            ones2 = consts.tile([128, 2], CDT, tag="c_ones2")
            nc.vector.memset(ones2, 1.0)
            ones70x128 = consts.tile([S2, 128], CDT, tag="c_ones70")
            nc.vector.memset(ones70x128, 1.0)
            onesf = consts.tile([1, BL], CDT, tag="c_onesf")
            nc.vector.memset(onesf, 1.0)
            cvecsb = consts.tile([2, 4], F32, tag="c_cvec")
            nc.sync.dma_start(out=cvecsb, in_=cvecd[:])
            ecols = []
            for p in range(3):
                e = consts.tile([128, 3], CDT, tag=f"c_e{p}")
                nc.vector.memset(e, 0.0)
                nc.vector.memset(e[:, p : p + 1], 1.0)
                ecols.append(e)

            aw1sb = consts.tile([S, D0], CDT, tag="c_aw1")
            nc.sync.dma_start(out=aw1sb, in_=aw1d[:])
            aw2sb = consts.tile([S, H], CDT, tag="c_aw2")
            nc.sync.dma_start(out=aw2sb, in_=aw2d[:])
            b1t = [
                consts.tile([128, 1], F32, tag=f"c_b1_{hc}", name=f"b1t{hc}")
                for hc in range(2)
            ]
            b2t = [
                consts.tile([128, 1], F32, tag=f"c_b2_{hc}", name=f"b2t{hc}")
                for hc in range(2)
            ]
            for hc in range(2):
                nc.sync.dma_start(out=b1t[hc], in_=b1d[hc, :, :])
                nc.sync.dma_start(out=b2t[hc], in_=b2d[hc, :, :])
            epsb = consts.tile([128, 1], F32, tag="c_epsb")
            nc.vector.memset(epsb, EPS)
            owsb = consts.tile([3, C], CDT, tag="c_ow")
            nc.sync.dma_start(out=owsb, in_=owbd[0:3, :])
            obsb = consts.tile([1, C], CDT, tag="c_ob")
            nc.sync.dma_start(out=obsb, in_=owbd[3:4, :])

            # ---------------- load inputs into padded layout ----------------
            def make_pads(n_chunks, tagpfx, zero=True):
                tiles = []
                for cc in range(n_chunks):
                    t = padsp.tile([128, BL * SP], CDT, tag="pad",
                                   name=f"pad_{tagpfx}{cc}")
                    tiles.append(t)
                if zero:
                    zero_borders(tiles)
                return tiles

            def zero_borders(tiles):
                # interiors are fully overwritten; only the K-1 halo columns
                # around each sample need zeros
                for t in tiles:
                    tv = t.rearrange("p (s w) -> p s w", w=SP)
                    nc.vector.memset(tv[:, 0:1, 0 : KK - 1], 0.0)
                    nc.vector.memset(
                        t[:, (BL - 1) * SP + KK - 1 + S : BL * SP], 0.0
                    )
                    mid = bass.AP(
                        tensor=t.tensor,
                        offset=t.offset + KK - 1 + S,
                        ap=[t.ap[0], [SP, BL - 1], [1, 2 * (KK - 1)]],
                    )
                    nc.vector.memset(mid, 0.0)

            # x1/x2 arrive host-pre-padded: plain full-tile DMAs, no memset dep
            x1pad = make_pads(6, "x1", zero=False)
            x2pad = make_pads(6, "x2", zero=False)

            for pads, src in ((x1pad, x1d), (x2pad, x2d)):
                for cc in range(6):
                    nc.sync.dma_start(
                        out=pads[cc],
                        in_=src[cc * 128 : (cc + 1) * 128, :],
                    )

            # unpadded view helper: [128, ns, S] slice of a pad tile
            def unpad(t, s0=0, s1=BL, width=S):
                return t.rearrange("p (s w) -> p s w", w=SP)[
                    :, s0:s1, KK - 1 : KK - 1 + width
                ]

            # ---------------- LO0 / RO0 (sums over positions) ----------------
            LO0 = []
            RO0 = []
            for pads, acc in ((x1pad, LO0), (x2pad, RO0)):
                for cc in range(6):
                    t = lop.tile([128, BL], F32, tag="lo")
                    nc.vector.reduce_sum(t, unpad(pads[cc]), axis=mybir.AxisListType.X)
                    acc.append(t)

            # ---------------- generic attention stage ----------------
            # xun(cc, s0, s1) -> AP [P, (s1-s0), W]; returns attA [W, BL*W] (=[i,j])
            # and attB (=[j,i]) tiles in CDT.
            def attention(n_cc, W, xun, yun, segs, tagpfx):
                NW = BL * W
                # squares
                sqx = []
                sqy = []
                for un, acc in ((xun, sqx), (yun, sqy)):
                    for cc in range(n_cc):
                        t = sqp.tile([128, NW], CDT, tag="sq")
                        u = un(cc, 0, BL)
                        nc.vector.tensor_mul(
                            t.rearrange("p (s w) -> p s w", w=W), u, u
                        )
                        sqx_ = acc.append(t)
                # n-chains: psum [2, seg], both rows = n
                def nchain(sqtiles):
                    pss = []
                    for (o0, o1) in segs:
                        ps = psp.tile([2, o1 - o0], F32, tag="ps")
                        for cc in range(n_cc):
                            nc.tensor.matmul(
                                ps,
                                lhsT=ones2,
                                rhs=sqtiles[cc][:, o0:o1],
                                start=(cc == 0),
                                stop=(cc == n_cc - 1),
                            )
                        pss.append(ps)
                    return pss
                n1ps = nchain(sqx)
                n2ps = nchain(sqy)
                # aug tiles
                aug_al = augp.tile([2, NW], CDT, tag="aug")  # (n1, 1)
                aug_ar = augp.tile([2, NW], CDT, tag="aug")  # (-1/2, -n2/2)
                aug_bl = augp.tile([2, NW], CDT, tag="aug")  # (n2, 1)
                aug_br = augp.tile([2, NW], CDT, tag="aug")  # (-1/2, -n1/2)
                # rows built in one dual-op tensor_scalar per seg:
                # out = in * mulvec + addvec, with per-partition vectors
                # keep = (1,0)+(0,1); nhalf = (0,-1/2)+(-1/2,0)
                c_keep_m, c_keep_a = cvecsb[:, 0:1], cvecsb[:, 1:2]
                c_nh_m, c_nh_a = cvecsb[:, 2:3], cvecsb[:, 3:4]
                for si, (o0, o1) in enumerate(segs):
                    nc.scalar.activation(
                        aug_al[:, o0:o1], n1ps[si], AF.Identity,
                        bias=c_keep_a, scale=c_keep_m,
                    )
                    nc.scalar.activation(
                        aug_br[:, o0:o1], n1ps[si], AF.Identity,
                        bias=c_nh_a, scale=c_nh_m,
                    )
                    nc.scalar.activation(
                        aug_bl[:, o0:o1], n2ps[si], AF.Identity,
                        bias=c_keep_a, scale=c_keep_m,
                    )
                    nc.scalar.activation(
                        aug_ar[:, o0:o1], n2ps[si], AF.Identity,
                        bias=c_nh_a, scale=c_nh_m,
                    )
                # per-sample dist chains -> batched nonlinearity
                attA = attbp.tile([W, NW], CDT, tag="attb")
                attB = attbp.tile([W, NW], CDT, tag="attb")
                distA = distp.tile([W, NW], CDT, tag="dist")
                distB = distp.tile([W, NW], CDT, tag="dist")
                for s in range(BL):
                    sl = slice(s * W, (s + 1) * W)
                    for (lef, rig, al, ar, dst) in (
                        (xun, yun, aug_al, aug_ar, distA),
                        (yun, xun, aug_bl, aug_br, distB),
                    ):
                        ps = psp.tile([W, W], F32, tag="ps")
                        for cc in range(n_cc):
                            nc.tensor.matmul(
                                ps,
                                lhsT=lef(cc, s, s + 1),
                                rhs=rig(cc, s, s + 1),
                                start=(cc == 0),
                                stop=False,
                            )
                        nc.tensor.matmul(
                            ps, lhsT=al[:, sl], rhs=ar[:, sl], start=False, stop=True
                        )
                        # dist^2 = max(-2 * ps, 0), evacuated batched
                        nc.vector.tensor_scalar(
                            dst[:, sl], ps, -2.0, 0.0, op0=ALU.mult, op1=ALU.max
                        )
                hw2 = NW // 2
                for (dst, dist) in ((attA, distA), (attB, distB)):
                    for h in range(2):
                        sl = slice(h * hw2, (h + 1) * hw2)
                        t2 = distp.tile([W, hw2], F32, tag="distf")
                        # t2 = sqrt(dist^2) + 1   (eps negligible: dist^2 >> 0)
                        nc.scalar.activation(t2, dist[:, sl], AF.Sqrt, bias=epsb[:W, :])
                        nc.vector.tensor_scalar_add(t2, t2, 1.0)
                        t3 = distp.tile([W, hw2], F32, tag="distf")
                        nc.vector.reciprocal_approx_fast(t3, t2)
                        with nc.allow_low_precision(reason="att stored bf16"):
                            nc.vector.tensor_copy(dst[:, sl], t3)
                return attA, attB

            x1un = lambda cc, s0, s1: unpad(x1pad[cc], s0, s1)
            x2un = lambda cc, s0, s1: unpad(x2pad[cc], s0, s1)
            if STAGES >= 2:
                attA1, attB1 = attention(6, S, x1un, x2un, SEG1024, "a1")

            # ---------------- x1a / x2a (aW1-weighted) ----------------
            x1apad = make_pads(6, "x1a")
            x2apad = make_pads(6, "x2a")
            # x1a[d, (s,i)] = sum_j attB1[j, (s,i)] * aW1[j, d]
            for attX, dstpads, awsb, ncc in (
                ((attB1, x1apad, aw1sb, 6),
                 (attA1, x2apad, aw1sb, 6)) if STAGES >= 3 else ()
            ):
                for cc in range(ncc):
                    for half in range(2):
                        ps = psp.tile([128, 8 * S], F32, tag="ps")
                        nc.tensor.matmul(
                            ps,
                            lhsT=awsb[:, cc * 128 : (cc + 1) * 128],
                            rhs=attX[:, half * 8 * S : (half + 1) * 8 * S],
                            start=True,
                            stop=True,
                        )
                        dst = unpad(dstpads[cc], half * 8, (half + 1) * 8)
                        nc.vector.tensor_copy(dst, ps)

            # ---------------- load conv weights ----------------
            def load_w(wd, n_cc, pool):
                tiles = {}
                for hc in range(2):
                    for k in range(KK):
                        for i in range(2):
                            t = pool.tile([128, n_cc * 128], CDT, tag="w")
                            nc.sync.dma_start(
                                out=t.rearrange("p (c h) -> p c h", h=128),
                                in_=wd[hc, k, i, :, :].rearrange(
                                    "(c p) h -> p c h", p=128
                                ),
                            )
                            tiles[(hc, k, i)] = t
                return tiles

            w1t = load_w(w1d, 6, w1p) if STAGES >= 4 else None

            # ---------------- conv layer (generic) ----------------
            def conv(n_cc, ipads, apads, wt, bt, tagpfx):
                # ipads/apads: [side][cc] pad tiles; returns lc[side][hc]
                lc = [[None, None], [None, None]]
                for hc in range(2):
                    pss = {}
                    for side in range(2):
                        for gi, (s0, s1) in enumerate(CONV_GROUPS):
                            pss[(side, gi)] = psp.tile(
                                [128, (s1 - s0) * S2], F32, tag="ps",
                                name=f"convps{side}_{gi}",
                            )
                    nstat = KK * 2 * n_cc
                    idx = 0
                    for k in range(KK):
                        for i in range(2):
                            for cc in range(n_cc):
                                stat = wt[(hc, k, i)].rearrange(
                                    "p (c h) -> p c h", h=128
                                )[:, cc, :]
                                first = idx == 0
                                last = idx == nstat - 1
                                idx += 1
                                for side in range(2):
                                    src = (ipads if i == 0 else apads)[side][cc]
                                    srcv = src.rearrange("p (s w) -> p s w", w=SP)
                                    for gi, (s0, s1) in enumerate(CONV_GROUPS):
                                        nc.tensor.matmul(
                                            pss[(side, gi)],
                                            lhsT=stat,
                                            rhs=srcv[:, s0:s1, k : k + S2],
                                            start=first,
                                            stop=last,
                                        )
                    for side in range(2):
                        t = lcp.tile([128, BL * S2], CDT, tag="lc")
                        lc[side][hc] = t
                        for gi, (s0, s1) in enumerate(CONV_GROUPS):
                            nc.scalar.activation(
                                t[:, s0 * S2 : s1 * S2],
                                pss[(side, gi)],
                                AF.Tanh,
                                bias=bt[hc],
                            )
                return lc

            lc1 = (
                conv(6, [x1pad, x2pad], [x1apad, x2apad], w1t, b1t, "c1")
                if STAGES >= 4 else None
            )

            # ---------------- layer-1 att2 + pooling ----------------
            lcun = lambda tiles: (  # noqa: E731
                lambda cc, s0, s1: tiles[cc].rearrange("p (s w) -> p s w", w=S2)[
                    :, s0:s1, :
                ]
            )
            if STAGES < 5:
                raise _StageCut
            att2A, att2B = attention(
                2, S2, lcun(lc1[0]), lcun(lc1[1]), SEG1120, "a2"
            )

            # la (multiplies lc, from att2B) / ra (multiplies rc, from att2A),
            # broadcast over 128 partitions via all-ones stationary
            labc = lcp.tile([128, BL * S2], CDT, tag="lc")
            rabc = lcp.tile([128, BL * S2], CDT, tag="lc")
            for att2X, dst in ((att2B, labc), (att2A, rabc)):
                for (o0, o1) in SEG1120:
                    ps = psp.tile([128, o1 - o0], F32, tag="ps")
                    nc.tensor.matmul(
                        ps, lhsT=ones70x128, rhs=att2X[:, o0:o1], start=True, stop=True
                    )
                    nc.vector.tensor_copy(dst[:, o0:o1], ps)

            # w_pool -> LI1/RI1 padded for layer 2; lap/rap sums
            li1pad = make_pads(2, "li1")
            ri1pad = make_pads(2, "ri1")
            lap1, rap1 = [], []
            for side, (bc, dstpads, acc) in enumerate(
                ((labc, li1pad, lap1), (rabc, ri1pad, rap1))
            ):
                for hc in range(2):
                    lct = lc1[side][hc]
                    w = wtreep.tile([128, BL * S2], CDT, tag="wt")
                    nc.vector.tensor_mul(w, lct, bc)
                    wv = w.rearrange("p (s w) -> p s w", w=S2)
                    t1 = wtreep.tile([128, BL * S2], CDT, tag="wt")
                    t1v = t1.rearrange("p (s w) -> p s w", w=S2)
                    nc.vector.tensor_add(
                        t1v[:, :, 0:69], wv[:, :, 0:69], wv[:, :, 1:70]
                    )
                    t2 = wtreep.tile([128, BL * S2], CDT, tag="wt")
                    t2v = t2.rearrange("p (s w) -> p s w", w=S2)
                    nc.vector.tensor_add(
                        t2v[:, :, 0:67], t1v[:, :, 0:67], t1v[:, :, 2:69]
                    )
                    t3 = wtreep.tile([128, BL * S2], CDT, tag="wt")
                    t3v = t3.rearrange("p (s w) -> p s w", w=S2)
                    nc.vector.tensor_add(
                        t3v[:, :, 0:64], t2v[:, :, 0:64], t1v[:, :, 4:68]
                    )
                    dst = dstpads[hc].rearrange("p (s w) -> p s w", w=SP)[
                        :, :, KK - 1 : KK - 1 + S
                    ]
                    nc.vector.tensor_add(dst, t3v[:, :, 0:64], wv[:, :, 6:70])
                    # lap/rap: sum over the 70 positions
                    t = lop.tile([128, BL], F32, tag="lo")
                    nc.vector.reduce_sum(
                        t,
                        lct.rearrange("p (s w) -> p s w", w=S2),
                        axis=mybir.AxisListType.X,
                    )
                    acc.append(t)

            # ---------------- layer 2 ----------------
            if STAGES < 6:
                raise _StageCut
            li1un = lambda cc, s0, s1: unpad(li1pad[cc], s0, s1)
            ri1un = lambda cc, s0, s1: unpad(ri1pad[cc], s0, s1)
            attA2, attB2 = attention(2, S, li1un, ri1un, SEG1024, "aL2")

            la2pad = make_pads(2, "la2")
            ra2pad = make_pads(2, "ra2")
            for attX, dstpads in ((attB2, la2pad), (attA2, ra2pad)):
                for cc in range(2):
                    for half in range(2):
                        ps = psp.tile([128, 8 * S], F32, tag="ps")
                        nc.tensor.matmul(
                            ps,
                            lhsT=aw2sb[:, cc * 128 : (cc + 1) * 128],
                            rhs=attX[:, half * 8 * S : (half + 1) * 8 * S],
                            start=True,
                            stop=True,
                        )
                        nc.vector.tensor_copy(
                            unpad(dstpads[cc], half * 8, (half + 1) * 8), ps
                        )

            if STAGES < 7:
                raise _StageCut
            w2t = load_w(w2d, 2, w2p)
            lc2 = conv(2, [li1pad, ri1pad], [la2pad, ra2pad], w2t, b2t, "c2")

            lap2, rap2 = [], []
            for side, acc in ((0, lap2), (1, rap2)):
                for hc in range(2):
                    t = lop.tile([128, BL], F32, tag="lo")
                    nc.vector.reduce_sum(
                        t,
                        lc2[side][hc].rearrange("p (s w) -> p s w", w=S2),
                        axis=mybir.AxisListType.X,
                    )
                    acc.append(t)

            # ---------------- cosine sims + output ----------------
            if STAGES < 8:
                raise _StageCut
            ps_cos = psp.tile([3, 3 * BL], F32, tag="ps")
            pairs = [(LO0, RO0), (lap1, rap1), (lap2, rap2)]
            nmm = sum(len(L) for L, _ in pairs)
            mi = 0
            for p, (Ls, Rs) in enumerate(pairs):
                for cc in range(len(Ls)):
                    mov = smallp.tile([128, 3 * BL], CDT, tag="cosmov")
                    nc.vector.tensor_mul(mov[:, 0:BL], Ls[cc], Rs[cc])
                    nc.vector.tensor_mul(mov[:, BL : 2 * BL], Ls[cc], Ls[cc])
                    nc.vector.tensor_mul(mov[:, 2 * BL : 3 * BL], Rs[cc], Rs[cc])
                    nc.tensor.matmul(
                        ps_cos,
                        lhsT=ecols[p],
                        rhs=mov,
                        start=(mi == 0),
                        stop=(mi == nmm - 1),
                    )
                    mi += 1
            sq = smallp.tile([3, 2 * BL], F32, tag="costail")
            nc.scalar.activation(sq, ps_cos[:, BL : 3 * BL], AF.Sqrt)
            den = smallp.tile([3, BL], F32, tag="costail")
            nc.vector.tensor_mul(den, sq[:, 0:BL], sq[:, BL : 2 * BL])
            den2 = smallp.tile([3, BL], F32, tag="costail")
            nc.vector.tensor_scalar_add(den2, den, EPS)
            rec = smallp.tile([3, BL], F32, tag="costail")
            nc.vector.reciprocal(rec, den2)
            cosv = smallp.tile([3, BL], CDT, tag="cosv")
            nc.vector.tensor_mul(cosv, ps_cos[:, 0:BL], rec)
            ps_out = psp.tile([BL, C], F32, tag="ps")
            nc.tensor.matmul(ps_out, lhsT=cosv, rhs=owsb, start=True, stop=False)
            nc.tensor.matmul(ps_out, lhsT=onesf, rhs=obsb, start=False, stop=True)
            outsb = smallp.tile([BL, C], F32, tag="outsb")
            nc.vector.tensor_copy(outsb, ps_out)
            nc.sync.dma_start(out=outd[:], in_=outsb)
          except _StageCut:
            dummy = smallp.tile([BL, C], F32, tag="outsb", name="dummy_out")
            nc.vector.memset(dummy, 0.0)
            nc.sync.dma_start(out=outd[:], in_=dummy)

    nc.compile()
    return nc


_GRAPH = None


def get_graph():
    global _GRAPH
    if _GRAPH is None:
        _GRAPH = build_graph()
    return _GRAPH


def make_in_maps(premise, hypothesis, aW1, conv1_w, conv1_b, aW2, conv2_w,
                 conv2_b, out_w, out_b):
    premise = np.asarray(premise, np.float32)
    hypothesis = np.asarray(hypothesis, np.float32)
    w1r = np.ascontiguousarray(
        np.asarray(conv1_w, np.float32)
        .reshape(D0, KK, 2, 2, 128)
        .transpose(3, 1, 2, 0, 4)
    ).astype(NPDT)  # [hc, K, i, D0, 128]
    w2r = np.ascontiguousarray(
        np.asarray(conv2_w, np.float32)
        .reshape(H, KK, 2, 2, 128)
        .transpose(3, 1, 2, 0, 4)
    ).astype(NPDT)
    aw1 = np.asarray(aW1, np.float32).astype(NPDT)
    aw2 = np.asarray(aW2, np.float32).astype(NPDT)
    b1 = np.asarray(conv1_b, np.float32).reshape(2, 128, 1)
    b2 = np.asarray(conv2_b, np.float32).reshape(2, 128, 1)
    cvec = np.array([[1.0, 0.0, 0.0, -0.5], [0.0, 1.0, -0.5, 0.0]], np.float32)
    owb = np.concatenate(
        [np.asarray(out_w, np.float32), np.asarray(out_b, np.float32)[None, :]], 0
    ).astype(NPDT)
    in_maps = []
    for i in range(NCORES):
        sl = slice(i * BL, (i + 1) * BL)
        def prep_x(arr):
            xp = np.zeros((BL, SP, D0), np.float32)
            xp[:, KK - 1 : KK - 1 + S, :] = arr[sl]
            return np.ascontiguousarray(
                xp.transpose(2, 0, 1).reshape(D0, BL * SP)
            ).astype(NPDT)

        x1 = prep_x(premise)
        x2 = prep_x(hypothesis)
        in_maps.append(
            dict(x1=x1, x2=x2, w1=w1r, w2=w2r, aw1=aw1, aw2=aw2, b1=b1, b2=b2,
                 owb=owb, cvec=cvec)
        )
    return in_maps


def run(in_maps, trace=False):
    nc = get_graph()
    return run_bass_kernel_spmd(nc, in_maps, list(range(NCORES)), trace=trace)


def kernel(**inputs):
    in_maps = make_in_maps(**inputs)
    res = run(in_maps, trace=False)
    out = np.concatenate([res.results[i]["out"] for i in range(NCORES)], axis=0)
    return np.asarray(out, np.float32)
